# revision 3
# baseline (speedup 1.0000x reference)
"""Scalar LSTM (I=H=O=1), B=1024, T=16384, followed by pointwise Linear.

Data-parallel over batch across 8 NeuronCores (128 rows/core, one batch row
per SBUF partition). The sequential-in-T recurrence is solved with a Picard
(DEER) fixed-point iteration, fully parallel over T within each sweep:

    sweep k:  u_g  = P_g + h_prev          P_g = (w_ih/w_hh)_g * x  (cached)
              gate = act(w_hh_g * u_g + beta_g)      (ACT LUT, scale+bias)
              z    = i * g
              c    = scan: c_t = f_t * c_{t-1} + z_t (DVE hw scan, fp32 state)
              h    = o * tanh(c)

Key design points (vs the 2.78ms baseline):
  * h feedback reads are fully STALE (previous sweep, Jacobi) — numerically
    verified identical convergence to the in-place scheme (the exact c-scan
    chaining does the forward propagation). This removes the serial
    cross-chunk dependency; only the c-scan boundary chains chunks, so all
    bulk work runs at engine throughput instead of dependency latency.
  * fp16 storage for P/H/u: tensor_tensor adds and mults hit the DVE 2x
    mode; scan keeps fp32 internal state. K=4 sweeps -> nre ~2.6e-3.
  * P_g precomputed once (4 tensor_scalar during sweep 0, fused with the
    input DMA) instead of 4 scalar_tensor_tensor per sweep (stt has no
    fast mode).
  * Software pipeline with 2-iteration lookahead: iteration i emits
    adds/acts for chunk i, z/scan/tanh for chunk i-1, h-mult for chunk i-2.
    Engines: DVE = 3 adds + z + scan + hmul, Pool = 1 add (+ y on last
    sweep), ACT = 4 gate LUTs + tanh.  ACT-bound at ~5us/chunk.
  * c and y tiles live in PSUM (8 banks exactly), freeing SBUF for the
    128KB fp16 P cache.
"""

import os
import numpy as np

B, T = 1024, 16384
NCORES = 8
BC = B // NCORES          # 128 batch rows per core = SBUF partitions
C = int(os.environ.get("KERNEL_CHUNK", "1024"))  # time-chunk size
K = int(os.environ.get("KERNEL_SWEEPS", "4"))    # fixed-point sweeps
UBUFS = int(os.environ.get("KERNEL_UBUFS", "3"))
CPSUM = bool(int(os.environ.get("KERNEL_CPSUM", "1")))  # c/y tiles in PSUM
T_OVERRIDE = int(os.environ.get("KERNEL_T", "0"))  # debug: shrink T
if T_OVERRIDE:
    T = T_OVERRIDE
NCH = T // C

LAST_RESULTS = None       # test.py introspects this for exec_time_ns


def _build_program(wih, whh, beta, W00, b0):
    import concourse.bacc as bacc
    import concourse.mybir as mybir
    from concourse.tile import TileContext

    F32 = mybir.dt.float32
    F16 = mybir.dt.float16
    AF = mybir.ActivationFunctionType
    OP = mybir.AluOpType

    # Per-gate immediates. gate order (i, f, g, o); funcs (sig, sig, tanh, sig)
    funcs = [AF.Sigmoid, AF.Sigmoid, AF.Tanh, AF.Sigmoid]
    wt = [0.0] * 4
    for g in range(4):
        assert abs(whh[g]) > 1e-8 * max(1.0, abs(wih[g])), (
            "degenerate w_hh; w~ folding invalid"
        )
        wt[g] = float(wih[g] / whh[g])
    v = [float(whh[g]) for g in range(4)]
    bt = [float(beta[g]) for g in range(4)]

    nc = bacc.Bacc(None, target_bir_lowering=False)
    xin = nc.declare_dram_parameter("x", [BC, T], F32, isOutput=False)
    yout = nc.declare_dram_parameter("y", [BC, T], F32, isOutput=True)

    with TileContext(nc) as tc:
        with (
            tc.tile_pool(name="persist", bufs=1) as pp,
            tc.tile_pool(name="work", bufs=UBUFS) as wp,
            tc.tile_pool(name="xp", bufs=2) as xp,
            tc.tile_pool(name="cpool", bufs=2,
                         space="PSUM" if CPSUM else "SBUF") as cp,
            tc.tile_pool(name="ypool", bufs=2) as yp,  # DMA needs SBUF src
        ):
            P = [pp.tile([BC, T], F16, name=f"P{g}") for g in range(4)]
            H = pp.tile([BC, T + 1], F16)
            btile = pp.tile([BC, 4], F32)

            nc.vector.memset(H[:, 0:1], 0.0)
            for g in range(4):
                nc.vector.memset(btile[:, g:g + 1], bt[g])

            # Pipeline state: tile handles per in-flight chunk
            U = [None] * NCH       # [u0..u3] per chunk
            CT = [None] * NCH      # c tile per chunk

            for k in range(K):
                first = (k == 0)
                last = (k == K - 1)
                for i in range(NCH + 2):
                    # ---- stage A: chunk i loads/adds + activations ----
                    if i < NCH:
                        s, e = i * C, (i + 1) * C
                        u = [wp.tile([BC, C], F16, name=f"u{g}", tag=f"u{g}")
                             for g in range(4)]
                        U[i] = u
                        if first:
                            xt = xp.tile([BC, C], F32, name="xt", tag="xt")
                            nc.sync.dma_start(out=xt, in_=xin[:, s:e])
                            # P_g = wt_g * x   (f32 -> f16); 2 on DVE, 2 Pool
                            for g in range(4):
                                eng = nc.vector if g < 2 else nc.gpsimd
                                eng.tensor_scalar(
                                    out=P[g][:, s:e], in0=xt[:, :],
                                    scalar1=wt[g], scalar2=None, op0=OP.mult)
                            # gates directly from P (h == 0 on sweep 0)
                            for g in range(4):
                                nc.scalar.activation(
                                    out=u[g][:, :], in_=P[g][:, s:e],
                                    func=funcs[g],
                                    bias=btile[:, g:g + 1], scale=v[g])
                        else:
                            # u_g = P_g + h_stale ; u0-u2 on DVE, u3 on Pool
                            for g in range(4):
                                eng = nc.vector if g < 3 else nc.gpsimd
                                eng.tensor_tensor(
                                    out=u[g][:, :], in0=P[g][:, s:e],
                                    in1=H[:, s:e], op=OP.add)
                            for g in range(4):
                                nc.scalar.activation(
                                    out=u[g][:, :], in_=u[g][:, :],
                                    func=funcs[g],
                                    bias=btile[:, g:g + 1], scale=v[g])

                    # ---- stage B: chunk i-1 z / scan / tanh ----
                    j = i - 1
                    if 0 <= j < NCH:
                        u = U[j]
                        # z = i*g (overwrites i)
                        nc.vector.tensor_tensor(
                            out=u[0][:, :], in0=u[0][:, :], in1=u[2][:, :],
                            op=OP.mult)
                        c = cp.tile([BC, C], F32, name="c", tag="c")
                        init = 0.0 if j == 0 else CT[j - 1][:, C - 1:C]
                        nc.vector.tensor_tensor_scan(
                            out=c[:, :], data0=u[1][:, :], data1=u[0][:, :],
                            initial=init, op0=OP.mult, op1=OP.add)
                        CT[j] = c
                        # tanh(c) overwrites the dead g tile
                        nc.scalar.activation(out=u[2][:, :], in_=c[:, :],
                                             func=AF.Tanh)

                    # ---- stage C: chunk i-2 h-mult (+ output on last) ----
                    j2 = i - 2
                    if 0 <= j2 < NCH:
                        s2, e2 = j2 * C, (j2 + 1) * C
                        u = U[j2]
                        nc.vector.tensor_tensor(
                            out=H[:, s2 + 1:e2 + 1], in0=u[3][:, :],
                            in1=u[2][:, :], op=OP.mult)
                        if last:
                            yt = yp.tile([BC, C], F32, name="yt", tag="yt")
                            nc.gpsimd.tensor_scalar(
                                out=yt[:, :], in0=H[:, s2 + 1:e2 + 1],
                                scalar1=W00, scalar2=b0,
                                op0=OP.mult, op1=OP.add)
                            nc.sync.dma_start(out=yout[:, s2:e2], in_=yt)

    if not nc.is_finalized():
        nc.finalize()
    return nc


def kernel(x, w_ih, w_hh, b_ih, b_hh, W, b):
    global LAST_RESULTS
    from concourse.bass_utils import run_bass_kernel_spmd

    x2 = np.ascontiguousarray(np.asarray(x, dtype=np.float32).reshape(B, T))
    wih = np.asarray(w_ih, dtype=np.float64).reshape(4)
    whh = np.asarray(w_hh, dtype=np.float64).reshape(4)
    beta = (np.asarray(b_ih, dtype=np.float64).reshape(4)
            + np.asarray(b_hh, dtype=np.float64).reshape(4))
    W00 = float(np.asarray(W, dtype=np.float64).reshape(1)[0])
    b0 = float(np.asarray(b, dtype=np.float64).reshape(1)[0])

    nc = _build_program(wih, whh, beta, W00, b0)

    in_maps = [{"x": x2[kk * BC:(kk + 1) * BC]} for kk in range(NCORES)]
    trace = bool(int(os.environ.get("KERNEL_TRACE", "0")))
    res = run_bass_kernel_spmd(nc, in_maps, list(range(NCORES)), trace=trace)
    LAST_RESULTS = res
    y = np.concatenate([res.results[kk]["y"] for kk in range(NCORES)], axis=0)
    return y.reshape(B, T, 1).astype(np.float32)
